# revision 19
# baseline (speedup 1.0000x reference)
"""GQA attention block (RoPE + causal softmax + out-proj) on 8 TRN2 cores.

Sharding: 8 cores = 2 batches x 4 kv-pairs. Core c handles batch c//4 and
kv heads {2p, 2p+1} (p = c%4), i.e. q heads 6p..6p+5. Each core computes its
partial y^T = wo_slice^T @ attn_out^T; the host sums the 4 partials per batch
and transposes back.

Per-core layout: everything stays feature-major [d, s] so no on-device
transposes of large activations are needed:
  Q^T/K^T: [128d, s]   (projection emits them directly)
  scores come out transposed: [t, s] blocks from lhsT=K^T-slice, rhs=Q^T
  probs [t, s] feed AV directly with V in [t, dv] (via small PE transposes)
RoPE is applied in [d, s] form by permuting the head dim on the HOST to
[evens | odds]; the rotation becomes a partition-block swap (done with a PE
permutation matmul) plus elementwise mul/adds. The softmax scale is folded
into wq on the host. Softmax runs without max-subtraction (scores are O(10),
exp is safe in fp32): probs = exp(scores) * binary_causal_mask, row sums via
an all-ones matmul into PSUM; 1/l comes from a DVE fast reciprocal on the
PSUM row, broadcast to 128 partitions on the (otherwise idle) gpsimd engine.

The emission order software-pipelines windows so the PE never sits in a
phase with a serial matmul->exp->matmul ping-pong: per window j,
  S1: kv/v projections(j)   interleaved with  attention pair1(j-1)
  S2: q projections(j)      interleaved with  attention pair2(j-1)
  S3: out-projection(j-1)   interleaved with  attention pair0(j)
tail: pairs 1+2 of the last window as a 4-unit interleave, then its
out-projection. wo stays resident in SBUF (loaded once during window 0).

Weights / activations are pre-tiled on the host into the exact SBUF tile
layouts so every streaming DMA is one big contiguous descriptor.
"""

import math
from contextlib import ExitStack

import numpy as np
import ml_dtypes

import concourse.bass as bass
import concourse.mybir as mybir
import concourse.tile as tile
from concourse import bacc
from concourse.bass_utils import run_bass_kernel_spmd
from concourse.masks import make_identity

B, S, DIM = 2, 2048, 3072
NH, NKV, HD = 24, 8, 128
QT_PER_CORE = 6   # q head-tiles per core
KV_PER_CORE = 2   # kv heads per core
NDT = QT_PER_CORE + 2 * KV_PER_CORE  # 10 projection d-tiles
NKT = DIM // 128  # 24 contraction tiles
SW = 512          # s-window (matmul moving free dim)
NJ = S // SW      # 4 windows
NTT = S // 128    # 16 t-tiles
SCALE = 1.0 / math.sqrt(HD)

F32 = mybir.dt.float32
BF16 = mybir.dt.bfloat16
BF = ml_dtypes.bfloat16

_PERM = np.concatenate([np.arange(0, HD, 2), np.arange(1, HD, 2)])

# projection emission order: k0,k1,v0,v1,q0..q5 so K/V for the current
# window exist before any of its attention pairs start
DT_ORDER = [6, 7, 8, 9, 0, 1, 2, 3, 4, 5]


def _splice(a, b):
    """Evenly merge list b into list a (no emission)."""
    if not a:
        return list(b)
    out, bi = [], 0
    for i, x in enumerate(a):
        out.append(x)
        want = (i + 1) * len(b) // len(a)
        while bi < want:
            out.append(b[bi])
            bi += 1
    out.extend(b[bi:])
    return out


def _interleave(prim, sec):
    """Emit prim closures in order, spreading sec closures evenly between."""
    if not sec:
        for p in prim:
            p()
        return
    if not prim:
        for s in sec:
            s()
        return
    si = 0
    n, m = len(prim), len(sec)
    for pi, p in enumerate(prim):
        p()
        want = (pi + 1) * m // n
        while si < want:
            sec[si]()
            si += 1
    while si < m:
        sec[si]()
        si += 1


def _build_body(nc, tc, io, ctx):
    x4, w10, wo4 = io["x4"], io["w10"], io["wo4"]
    ropeC, ropeS, masks, swp, yT = (
        io["ropeC"], io["ropeS"], io["masks"], io["swp"], io["yT"])

    singles = ctx.enter_context(tc.tile_pool(name="singles", bufs=1))
    ps = ctx.enter_context(tc.tile_pool(name="ps", bufs=1, space=bass.MemorySpace.PSUM))
    xt_pool = ctx.enter_context(tc.tile_pool(name="xtp", bufs=2))
    w_pool = ctx.enter_context(tc.tile_pool(name="wtp", bufs=5))
    raw_pool = ctx.enter_context(tc.tile_pool(name="rawp", bufs=3))
    qT_pool = ctx.enter_context(tc.tile_pool(name="qTp", bufs=8))
    probs_pool = ctx.enter_context(tc.tile_pool(name="prp", bufs=8))
    small_pool = ctx.enter_context(tc.tile_pool(name="smp", bufs=2))
    out_pool = ctx.enter_context(tc.tile_pool(name="otp", bufs=8))
    y_pool = ctx.enter_context(tc.tile_pool(name="yp", bufs=3))
    acc_pool = ctx.enter_context(tc.tile_pool(name="accp", bufs=2))

    # constants / persistent state (const DMAs ride the gpsimd queue so they
    # don't delay the first x/weight loads on sync/scalar)
    ropeC_sb = singles.tile([128, S], F32, tag="ropeC", name="ropeC_sb")
    ropeS_sb = singles.tile([128, S], F32, tag="ropeS", name="ropeS_sb")
    masks_sb = singles.tile([128, 4, SW], BF16, tag="masks", name="masks_sb")
    swp_sb = singles.tile([128, 128], BF16, tag="swp", name="swp_sb")
    ident = singles.tile([128, 128], F32, tag="ident", name="ident")
    ones_t = singles.tile([128, 1], BF16, tag="ones_t", name="ones_t")
    ones_r = singles.tile([128, 1], mybir.dt.float32r, tag="ones_r",
                          name="ones_r")
    ones_mat = singles.tile([128, 128], mybir.dt.float32r, tag="ones_mat",
                            name="ones_mat")
    ones_mat0 = singles.tile([128, 128], F32, tag="ones_mat0", name="ones_mat0")
    nc.gpsimd.dma_start(out=ropeC_sb, in_=ropeC[:])
    nc.gpsimd.dma_start(out=ropeS_sb, in_=ropeS[:])
    nc.gpsimd.dma_start(out=masks_sb, in_=masks[:])
    nc.gpsimd.dma_start(out=swp_sb, in_=swp[:])
    make_identity(nc, ident)
    nc.vector.memset(ones_t, 1.0)
    nc.vector.memset(ones_r[:, :].bitcast(F32), 1.0)
    nc.vector.memset(ones_mat0, 1.0)
    nc.scalar.copy(out=ones_mat, in_=ones_mat0)

    KT_sb = [singles.tile([128, S], BF16, tag=f"KT{g}", name=f"KT{g}")
             for g in range(KV_PER_CORE)]
    V_sb = [singles.tile([128, NTT, 128], BF16, tag=f"V{g}", name=f"V{g}")
            for g in range(KV_PER_CORE)]
    # resident out-projection weights, loaded once during window 0
    wo_sb = singles.tile([128, NKT, QT_PER_CORE, 128], BF16, tag="wo_sb",
                         name="wo_sb")

    qTj = [[None] * QT_PER_CORE for _ in range(NJ)]
    outTj = [[None] * QT_PER_CORE for _ in range(NJ)]
    xt_tiles = [None] * NJ
    wt_tiles = {}

    # ---- DMA issue helpers ----
    def issue_lead_in():
        # first weight tile in fine chunks on sync so the first matmuls can
        # start as soon as the first k-slices land; x window 0 on scalar
        wt = w_pool.tile([128, NKT, 128], BF16, name="wt")
        wt_tiles[(0, 0)] = wt
        for k0, k1 in [(0, 2), (2, 6), (6, 12), (12, 24)]:
            nc.sync.dma_start(out=wt[:, k0:k1, :], in_=w10[DT_ORDER[0], :, k0:k1, :])
        xt_tiles[0] = xt_pool.tile([128, NKT, SW], BF16, name="xt")
        xt = xt_tiles[0]
        xsl = [(0, 1), (1, 2), (2, 4), (4, 6), (6, 9), (9, 12),
               (12, 16), (16, 20), (20, 24)]
        for k0, k1 in xsl:
            nc.scalar.dma_start(out=xt[:, k0:k1, :], in_=x4[0, :, k0:k1, :])
        issue_wt(0, 1)()
        issue_wt(0, 2)()

    def issue_xt(j):
        def run():
            xt_tiles[j] = xt_pool.tile([128, NKT, SW], BF16, name="xt")
            xt = xt_tiles[j]
            for k0, k1 in [(0, 6), (6, 12), (12, 18), (18, 24)]:
                nc.sync.dma_start(out=xt[:, k0:k1, :], in_=x4[j, :, k0:k1, :])
        return run

    def issue_wt(j, p):
        def run():
            if (j, p) in wt_tiles:
                return
            wt = w_pool.tile([128, NKT, 128], BF16, name="wt")
            wt_tiles[(j, p)] = wt
            nc.sync.dma_start(out=wt, in_=w10[DT_ORDER[p]])
        return run

    def wo_load_closures():
        cls = []
        for d in range(NKT):
            def run(d=d):
                eng = nc.scalar if d % 2 == 0 else nc.sync
                eng.dma_start(out=wo_sb[:, d], in_=wo4[d])
            cls.append(run)
        return cls

    # ---- projection ----
    def post_qk(j, dt, raw, sw_ps):
        jw = bass.ts(j, SW)

        def run():
            if dt < 6:
                qt = qT_pool.tile([128, SW], BF16, name="qt")
                qTj[j][dt] = qt
                dest = qt
            else:
                dest = KT_sb[dt - 6][:, jw]
            nc.vector.tensor_mul(dest, raw, ropeC_sb[:, jw])
            t2 = raw_pool.tile([128, SW], BF16, name="t2")
            nc.vector.tensor_mul(t2, sw_ps, ropeS_sb[:, jw])
            nc.vector.tensor_add(dest, dest, t2)
        return run

    def post_v(j, dt, vraw):
        def run():
            g = dt - 8
            tp = ps.tile([128, SW], F32, tag="pp", bufs=2, name="tp")
            for rr in range(4):
                nc.tensor.transpose(tp[:, bass.ts(rr, 128)],
                                    vraw[:, bass.ts(rr, 128)], ident)
            nc.scalar.copy(out=V_sb[g][:, 4 * j:4 * j + 4, :],
                           in_=tp.rearrange("p (r t) -> p r t", r=4))
        return run

    def proj_closures(j, positions):
        """Closures for projection d-tiles at the given DT_ORDER positions."""
        cls = []
        for p in positions:
            dt = DT_ORDER[p]
            # prefetch weights two positions ahead (positions 0/1 of this
            # window were issued by the previous window / lead-in)
            if p + 2 < NDT:
                cls.append(issue_wt(j, p + 2))
            elif j + 1 < NJ:
                cls.append(issue_wt(j + 1, p + 2 - NDT))

            # matmul chunks of 4 k-tiles; PSUM tile created by first chunk
            st = {}

            def mk_chunk(p=p, k0=0, k1=0, st=st):
                def run():
                    if k0 == 0:
                        st['pp'] = ps.tile([128, SW], F32, tag="pp", bufs=2,
                                           name="pp")
                    wt = wt_tiles[(j, p)]
                    xt = xt_tiles[j]
                    for k in range(k0, k1):
                        nc.tensor.matmul(st['pp'], wt[:, k, :], xt[:, k, :],
                                         start=(k == 0), stop=(k == NKT - 1),
                                         skip_group_check=True)
                return run

            for k0 in range(0, NKT, 4):
                cls.append(mk_chunk(p=p, k0=k0, k1=k0 + 4, st=st))

            def mk_evac(dt=dt, st=st):
                def run():
                    if dt >= 8:
                        vraw = raw_pool.tile([128, SW], F32, name="vraw")
                        nc.any.tensor_copy(out=vraw, in_=st['pp'])
                        st['post'] = post_v(j, dt, vraw)
                    else:
                        raw = raw_pool.tile([128, SW], BF16, name="raw")
                        nc.any.tensor_copy(out=raw, in_=st['pp'])
                        sw_ps = ps.tile([128, SW], F32, tag="pp", bufs=2,
                                        name="sw_ps")
                        nc.tensor.matmul(sw_ps, swp_sb, raw, start=True,
                                         stop=True, skip_group_check=True)
                        st['post'] = post_qk(j, dt, raw, sw_ps)
                return run

            cls.append(mk_evac(dt=dt, st=st))
            # deferred post of this dt runs one position later; stash it
            cls.append(('post_marker', st))
        # resolve post markers: run each dt's post inside the NEXT dt's block
        out, pending = [], []
        for c in cls:
            if isinstance(c, tuple):
                pending.append(c[1])
                if len(pending) > 1:
                    stp = pending.pop(0)
                    out.append(lambda stp=stp: stp['post']())
            else:
                out.append(c)
        if pending:
            stp = pending.pop(0)
            out.append(lambda stp=stp: stp['post']())
        return out

    # ---- attention ----
    def pair_closures(j, units, l_on_pe=False, av_tags=None):
        n = len(units)
        if av_tags is None:
            av_tags = ("av",) * n
        lrows = tuple(32 * a for a in range(n))
        nlast = 4 * j + 3
        st = {}
        cls = []

        def start():
            if l_on_pe:
                st['lp'] = ps.tile([128, SW], F32, tag="l", bufs=1, name="lp")
            else:
                st['acc'] = [acc_pool.tile([128, SW], mybir.dt.float32r,
                                           name="acc") for _ in range(n)]
            st['avs'] = [ps.tile([128, SW], F32, tag=av_tags[a], bufs=2,
                                 name=f"av{a}") for a in range(n)]
            st['pr'] = {}
        cls.append(start)

        def _lo(i):
            # valid s-range of diagonal tile r starts at 128*r (causal)
            r = i - 4 * j
            return 128 * r if r > 0 else 0

        def emit_scores(i, a):
            u = units[a]
            g = u // 3
            lo = _lo(i)
            sc = ps.tile([128, SW], F32, tag="sy", bufs=3, name="sc")
            nc.tensor.matmul(sc[:, lo:], KT_sb[g][:, bass.ts(i, 128)],
                             qTj[j][u][:, lo:], start=True, stop=True,
                             skip_group_check=True)
            pr = probs_pool.tile([128, SW], BF16, name="pr")
            nc.scalar.activation(out=pr[:, lo:], in_=sc[:, lo:],
                                 func=mybir.ActivationFunctionType.Exp)
            if i >= 4 * j:
                # only the first 128 columns of the valid range straddle the
                # diagonal; the rest is fully visible
                nc.vector.tensor_mul(pr[:, lo:lo + 128], pr[:, lo:lo + 128],
                                     masks_sb[:, i - 4 * j, lo:lo + 128])
            st['pr'][(i, a)] = pr
            if not l_on_pe:
                if i == 0:
                    nc.scalar.copy(out=st['acc'][a], in_=pr)
                else:
                    nc.gpsimd.tensor_add(st['acc'][a][:, lo:],
                                         st['acc'][a][:, lo:], pr[:, lo:])

        def emit_lav(i, a):
            u = units[a]
            g = u // 3
            lo = _lo(i)
            pr = st['pr'].pop((i, a))
            if l_on_pe:
                row = lrows[a]
                nc.tensor.matmul(st['lp'][row:row + 1, lo:], ones_t,
                                 pr[:, lo:], start=(i == 0), stop=(i == nlast),
                                 skip_group_check=True)
            nc.tensor.matmul(st['avs'][a][:, lo:], V_sb[g][:, i, :],
                             pr[:, lo:], start=(i == 0), stop=(i == nlast),
                             skip_group_check=True)

        def mk_step(i):
            def run():
                for a in range(n):
                    emit_scores(i, a)
                if i >= 1:
                    for a in range(n):
                        emit_lav(i - 1, a)
                if i == nlast:
                    for a in range(n):
                        emit_lav(i, a)
            return run

        for i in range(4 * j + 4):
            cls.append(mk_step(i))

        def mk_fin(a):
            def run():
                u = units[a]
                l_sbp = small_pool.tile([1, SW], mybir.dt.float32r, tag="rrow",
                                        name="l_sbp")
                if l_on_pe:
                    row = lrows[a]
                    nc.scalar.copy(out=l_sbp, in_=st['lp'][row:row + 1, :])
                else:
                    lc = ps.tile([128, SW], F32, tag="l", bufs=1, name="lc")
                    nc.tensor.matmul(lc[0:1, :], ones_r, st['acc'][a],
                                     start=True, stop=True,
                                     skip_group_check=True)
                    nc.scalar.copy(out=l_sbp, in_=lc[0:1, :])
                rb = ps.tile([128, SW], F32, tag="sy", bufs=3, name="rb")
                nc.tensor.matmul(rb, ones_mat[0:1, :], l_sbp, start=True,
                                 stop=True, skip_group_check=True)
                rbs = small_pool.tile([128, SW], F32, tag="rbs", name="rbs")
                nc.vector.reciprocal_approx_fast(out=rbs, in_=rb)
                ot = out_pool.tile([128, SW], BF16, name="ot")
                nc.vector.tensor_mul(ot, st['avs'][a], rbs)
                outTj[j][u] = ot
            return run

        for a in range(n):
            cls.append(mk_fin(a))
        return cls

    # ---- out-projection ----
    def outproj_closures(j):
        cls = []
        for d in range(NKT):
            def run(d=d):
                yp = ps.tile([128, SW], F32, tag="sy", bufs=3, name="yp")
                for u in range(QT_PER_CORE):
                    nc.tensor.matmul(yp, wo_sb[:, d, u, :], outTj[j][u],
                                     start=(u == 0),
                                     stop=(u == QT_PER_CORE - 1),
                                     skip_group_check=True)
                ys = y_pool.tile([128, SW], BF16, name="ys")
                nc.vector.tensor_copy(out=ys, in_=yp)
                nc.scalar.dma_start(out=yT[bass.ts(d, 128), bass.ts(j, SW)],
                                    in_=ys)
            cls.append(run)
        return cls

    # ---- schedule ----
    issue_lead_in()
    wo_cls = wo_load_closures()
    for j in range(NJ):
        kv_cls = proj_closures(j, [0, 1, 2, 3])
        qa_cls = proj_closures(j, [4, 5, 6])
        qb_cls = proj_closures(j, [7, 8, 9])
        s2_extra = [issue_xt(j + 1)] if j + 1 < NJ else []
        if j == 0:
            _interleave(kv_cls, [])
            for c in qa_cls:
                c()
            _interleave(qb_cls + s2_extra, pair_closures(0, (0, 1)))
            _interleave(wo_cls[:8], pair_closures(0, (2, 3)))
        else:
            prim1 = _splice(kv_cls, wo_cls[8:]) if j == 1 else kv_cls
            _interleave(prim1, pair_closures(j - 1, (4, 5)))
            for c in qa_cls:
                c()
            _interleave(qb_cls + s2_extra, pair_closures(j, (0, 1)))
            _interleave(outproj_closures(j - 1), pair_closures(j, (2, 3)))
    # tail: last window's remaining pair, then its out-projection
    _interleave([], pair_closures(NJ - 1, (4, 5), l_on_pe=True))
    for c in outproj_closures(NJ - 1):
        c()


def build_nc():
    nc = bacc.Bacc("TRN2", target_bir_lowering=False, debug=False, num_devices=8)
    io = {
        "x4": nc.dram_tensor("x4", [NJ, 128, NKT, SW], BF16, kind="ExternalInput"),
        "w10": nc.dram_tensor("w10", [NDT, 128, NKT, 128], BF16, kind="ExternalInput"),
        "wo4": nc.dram_tensor("wo4", [NKT, 128, QT_PER_CORE, 128], BF16,
                              kind="ExternalInput"),
        "ropeC": nc.dram_tensor("ropeC", [HD, S], F32, kind="ExternalInput"),
        "ropeS": nc.dram_tensor("ropeS", [HD, S], F32, kind="ExternalInput"),
        "masks": nc.dram_tensor("masks", [128, 4, SW], BF16, kind="ExternalInput"),
        "swp": nc.dram_tensor("swp", [128, 128], BF16, kind="ExternalInput"),
        "yT": nc.dram_tensor("yT", [DIM, S], BF16, kind="ExternalOutput"),
    }
    with tile.TileContext(nc) as tc:
        with ExitStack() as ctx:
            _build_body(nc, tc, io, ctx)
    nc.compile()
    return nc


_NC = None


def _get_nc():
    global _NC
    if _NC is None:
        _NC = build_nc()
    return _NC


def make_in_maps(x, wq, wk, wv, wo, freqs_cos, freqs_sin):
    x = np.asarray(x, np.float32)
    wq = np.asarray(wq, np.float32)
    wk = np.asarray(wk, np.float32)
    wv = np.asarray(wv, np.float32)
    wo = np.asarray(wo, np.float32)
    cos = np.asarray(freqs_cos, np.float32)
    sin = np.asarray(freqs_sin, np.float32)

    wq_p = (wq.reshape(DIM, NH, HD)[:, :, _PERM] * SCALE).astype(BF)
    wk_p = wk.reshape(DIM, NKV, HD)[:, :, _PERM].astype(BF)
    wv_r = wv.reshape(DIM, NKV, HD).astype(BF)
    wo_r = wo.reshape(NH, HD, DIM)

    ropeC = np.ascontiguousarray(np.concatenate([cos.T, cos.T], 0))
    ropeS = np.ascontiguousarray(np.concatenate([-sin.T, sin.T], 0))

    tt = np.arange(128)[:, None]
    ss = np.arange(SW)[None, :]
    # [128, 4, SW] with masks[:, r, :] the r-th diagonal-block pattern
    masks = np.stack([(128 * r + tt <= ss) for r in range(4)], axis=1).astype(BF)

    swp = np.zeros((128, 128), BF)
    swp[np.arange(128), (np.arange(128) + 64) % 128] = 1.0

    in_maps = []
    for c in range(8):
        b, p = divmod(c, 4)
        # per-core weight slices in on-chip tile layout
        wq_c = wq_p[:, 6 * p:6 * p + 6, :]          # [DIM, 6, 128]
        wk_c = wk_p[:, 2 * p:2 * p + 2, :]          # [DIM, 2, 128]
        wv_c = wv_r[:, 2 * p:2 * p + 2, :]          # [DIM, 2, 128]
        # w10[dt] = [128p, 24k, 128d] with DIM rows split as (k, p)
        wcat = np.concatenate([wq_c, wk_c, wv_c], axis=1)   # [DIM, 10, 128]
        w10 = np.ascontiguousarray(
            wcat.reshape(NKT, 128, NDT, HD).transpose(2, 1, 0, 3))
        # wo4[d] = [128p(dv), 6u, 128dd]; wo rows are (u, p)
        wo_c = wo_r[6 * p:6 * p + 6]                 # [6, 128, DIM]
        wo4 = np.ascontiguousarray(
            wo_c.reshape(QT_PER_CORE, HD, NKT, 128).transpose(2, 1, 0, 3)).astype(BF)
        # x4[j] = [128p, 24k, 512s]
        xT_b = x[b].T                                 # [DIM, S]
        x4 = np.ascontiguousarray(
            xT_b.reshape(NKT, 128, NJ, SW).transpose(2, 1, 0, 3)).astype(BF)
        in_maps.append({
            "x4": x4,
            "w10": w10,
            "wo4": wo4,
            "ropeC": ropeC,
            "ropeS": ropeS,
            "masks": masks,
            "swp": swp,
        })
    return in_maps


def gather(results):
    y = np.empty((B, S, DIM), np.float32)
    for b in range(B):
        acc = results[4 * b]["yT"].astype(np.float32)
        for p in range(1, 4):
            acc = acc + results[4 * b + p]["yT"].astype(np.float32)
        y[b] = acc.T
    return y


def kernel(x, wq, wk, wv, wo, freqs_cos, freqs_sin, **run_kwargs):
    nc = _get_nc()
    in_maps = make_in_maps(x, wq, wk, wv, wo, freqs_cos, freqs_sin)
    res = run_bass_kernel_spmd(nc, in_maps, core_ids=list(range(8)), **run_kwargs)
    out = gather(res.results)
    if run_kwargs:
        return out, res
    return out


# revision 20
# speedup vs baseline: 1.2101x; 1.2101x over previous
"""GQA attention block (RoPE + causal softmax + out-proj) on 8 TRN2 cores.

Sharding: 8 cores = 2 batches x 4 kv-pairs. Core c handles batch c//4 and
kv heads {2p, 2p+1} (p = c%4), i.e. q heads 6p..6p+5. Each core computes its
partial y^T = wo_slice^T @ attn_out^T; the host sums the 4 partials per batch
and transposes back.

Per-core layout: everything stays feature-major [d, s] so no on-device
transposes of large activations are needed:
  Q^T/K^T: [128d, s]   (projection emits them directly)
  scores come out transposed: [t, s] blocks from lhsT=K^T-slice, rhs=Q^T
  probs [t, s] feed AV directly with V in [t, dv] (via small PE transposes)
RoPE is applied in [d, s] form by permuting the head dim on the HOST to
[evens | odds]; the rotation becomes a partition-block swap (done with a PE
permutation matmul) plus elementwise mul/adds. The softmax scale is folded
into wq on the host. Softmax runs without max-subtraction (scores are O(10),
exp is safe in fp32): probs = exp(scores) * binary_causal_mask, row sums via
an all-ones matmul into PSUM; 1/l comes from a DVE fast reciprocal on the
PSUM row, broadcast to 128 partitions on the (otherwise idle) gpsimd engine.

The emission order software-pipelines windows so the PE never sits in a
phase with a serial matmul->exp->matmul ping-pong: per window j,
  S1: kv/v projections(j)   interleaved with  attention pair1(j-1)
  S2: q projections(j)      interleaved with  attention pair2(j-1)
  S3: out-projection(j-1)   interleaved with  attention pair0(j)
tail: pairs 1+2 of the last window as a 4-unit interleave, then its
out-projection. wo stays resident in SBUF (loaded once during window 0).

Weights / activations are pre-tiled on the host into the exact SBUF tile
layouts so every streaming DMA is one big contiguous descriptor.
"""

import math
from contextlib import ExitStack

import numpy as np
import ml_dtypes

import concourse.bass as bass
import concourse.mybir as mybir
import concourse.tile as tile
from concourse import bacc
from concourse.bass_utils import run_bass_kernel_spmd
from concourse.masks import make_identity

B, S, DIM = 2, 2048, 3072
NH, NKV, HD = 24, 8, 128
QT_PER_CORE = 6   # q head-tiles per core
KV_PER_CORE = 2   # kv heads per core
NDT = QT_PER_CORE + 2 * KV_PER_CORE  # 10 projection d-tiles
NKT = DIM // 128  # 24 contraction tiles
SW = 512          # s-window (matmul moving free dim)
NJ = S // SW      # 4 windows
NTT = S // 128    # 16 t-tiles
SCALE = 1.0 / math.sqrt(HD)

F32 = mybir.dt.float32
BF16 = mybir.dt.bfloat16
BF = ml_dtypes.bfloat16

_PERM = np.concatenate([np.arange(0, HD, 2), np.arange(1, HD, 2)])

# projection emission order: k0,k1,v0,v1,q0..q5 so K/V for the current
# window exist before any of its attention pairs start
DT_ORDER = [6, 7, 8, 9, 0, 1, 2, 3, 4, 5]


def _splice(a, b):
    """Evenly merge list b into list a (no emission)."""
    if not a:
        return list(b)
    out, bi = [], 0
    for i, x in enumerate(a):
        out.append(x)
        want = (i + 1) * len(b) // len(a)
        while bi < want:
            out.append(b[bi])
            bi += 1
    out.extend(b[bi:])
    return out


def _interleave(prim, sec):
    """Emit prim closures in order, spreading sec closures evenly between."""
    if not sec:
        for p in prim:
            p()
        return
    if not prim:
        for s in sec:
            s()
        return
    si = 0
    n, m = len(prim), len(sec)
    for pi, p in enumerate(prim):
        p()
        want = (pi + 1) * m // n
        while si < want:
            sec[si]()
            si += 1
    while si < m:
        sec[si]()
        si += 1


def _build_body(nc, tc, io, ctx):
    x4, w10, wo4 = io["x4"], io["w10"], io["wo4"]
    ropeC, ropeS, masks, swp, yT = (
        io["ropeC"], io["ropeS"], io["masks"], io["swp"], io["yT"])

    singles = ctx.enter_context(tc.tile_pool(name="singles", bufs=1))
    ps = ctx.enter_context(tc.tile_pool(name="ps", bufs=1, space=bass.MemorySpace.PSUM))
    xt_pool = ctx.enter_context(tc.tile_pool(name="xtp", bufs=2))
    w_pool = ctx.enter_context(tc.tile_pool(name="wtp", bufs=5))
    raw_pool = ctx.enter_context(tc.tile_pool(name="rawp", bufs=3))
    qT_pool = ctx.enter_context(tc.tile_pool(name="qTp", bufs=8))
    probs_pool = ctx.enter_context(tc.tile_pool(name="prp", bufs=8))
    small_pool = ctx.enter_context(tc.tile_pool(name="smp", bufs=2))
    out_pool = ctx.enter_context(tc.tile_pool(name="otp", bufs=8))
    y_pool = ctx.enter_context(tc.tile_pool(name="yp", bufs=3))
    acc_pool = ctx.enter_context(tc.tile_pool(name="accp", bufs=2))

    # constants / persistent state (const DMAs ride the gpsimd queue so they
    # don't delay the first x/weight loads on sync/scalar)
    ropeC_sb = singles.tile([128, S], F32, tag="ropeC", name="ropeC_sb")
    ropeS_sb = singles.tile([128, S], F32, tag="ropeS", name="ropeS_sb")
    masks_sb = singles.tile([128, 4, SW], BF16, tag="masks", name="masks_sb")
    swp_sb = singles.tile([128, 128], BF16, tag="swp", name="swp_sb")
    ident = singles.tile([128, 128], F32, tag="ident", name="ident")
    ones_t = singles.tile([128, 1], BF16, tag="ones_t", name="ones_t")
    ones_r = singles.tile([128, 1], mybir.dt.float32r, tag="ones_r",
                          name="ones_r")
    ones_mat = singles.tile([128, 128], BF16, tag="ones_mat",
                            name="ones_mat")
    nc.gpsimd.dma_start(out=ropeC_sb, in_=ropeC[:])
    nc.gpsimd.dma_start(out=ropeS_sb, in_=ropeS[:])
    nc.gpsimd.dma_start(out=masks_sb, in_=masks[:])
    nc.gpsimd.dma_start(out=swp_sb, in_=swp[:])
    make_identity(nc, ident)
    nc.vector.memset(ones_t, 1.0)
    nc.vector.memset(ones_r[:, :].bitcast(F32), 1.0)
    nc.vector.memset(ones_mat, 1.0)

    KT_sb = [singles.tile([128, S], BF16, tag=f"KT{g}", name=f"KT{g}")
             for g in range(KV_PER_CORE)]
    V_sb = [singles.tile([128, NTT, 128], BF16, tag=f"V{g}", name=f"V{g}")
            for g in range(KV_PER_CORE)]
    # resident out-projection weights, loaded once during window 0
    wo_sb = singles.tile([128, NKT, QT_PER_CORE, 128], BF16, tag="wo_sb",
                         name="wo_sb")

    qTj = [[None] * QT_PER_CORE for _ in range(NJ)]
    outTj = [[None] * QT_PER_CORE for _ in range(NJ)]
    xt_tiles = [None] * NJ
    wt_tiles = {}

    # ---- DMA issue helpers ----
    def issue_lead_in():
        # first weight tile in fine chunks on sync so the first matmuls can
        # start as soon as the first k-slices land; x window 0 on scalar
        wt = w_pool.tile([128, NKT, 128], BF16, name="wt")
        wt_tiles[(0, 0)] = wt
        for k0, k1 in [(0, 2), (2, 6), (6, 12), (12, 24)]:
            nc.sync.dma_start(out=wt[:, k0:k1, :], in_=w10[DT_ORDER[0], :, k0:k1, :])
        xt_tiles[0] = xt_pool.tile([128, NKT, SW], BF16, name="xt")
        xt = xt_tiles[0]
        xsl = [(0, 1), (1, 2), (2, 4), (4, 6), (6, 9), (9, 12),
               (12, 16), (16, 20), (20, 24)]
        for k0, k1 in xsl:
            nc.scalar.dma_start(out=xt[:, k0:k1, :], in_=x4[0, :, k0:k1, :])
        issue_wt(0, 1)()
        issue_wt(0, 2)()

    def issue_xt(j):
        def run():
            xt_tiles[j] = xt_pool.tile([128, NKT, SW], BF16, name="xt")
            xt = xt_tiles[j]
            for k0, k1 in [(0, 6), (6, 12), (12, 18), (18, 24)]:
                nc.sync.dma_start(out=xt[:, k0:k1, :], in_=x4[j, :, k0:k1, :])
        return run

    def issue_wt(j, p):
        def run():
            if (j, p) in wt_tiles:
                return
            wt = w_pool.tile([128, NKT, 128], BF16, name="wt")
            wt_tiles[(j, p)] = wt
            nc.sync.dma_start(out=wt, in_=w10[DT_ORDER[p]])
        return run

    def wo_load_closures():
        cls = []
        for d in range(NKT):
            def run(d=d):
                eng = nc.scalar if d % 2 == 0 else nc.sync
                eng.dma_start(out=wo_sb[:, d], in_=wo4[d])
            cls.append(run)
        return cls

    # ---- projection ----
    def post_qk(j, dt, raw, sw_ps):
        jw = bass.ts(j, SW)

        def run():
            if dt < 6:
                qt = qT_pool.tile([128, SW], BF16, name="qt")
                qTj[j][dt] = qt
                dest = qt
            else:
                dest = KT_sb[dt - 6][:, jw]
            nc.vector.tensor_mul(dest, raw, ropeC_sb[:, jw])
            t2 = raw_pool.tile([128, SW], BF16, name="t2")
            nc.vector.tensor_mul(t2, sw_ps, ropeS_sb[:, jw])
            nc.vector.tensor_add(dest, dest, t2)
        return run

    def post_v(j, dt, vraw):
        def run():
            g = dt - 8
            tp = ps.tile([128, SW], F32, tag="pp", bufs=2, name="tp")
            for rr in range(4):
                nc.tensor.transpose(tp[:, bass.ts(rr, 128)],
                                    vraw[:, bass.ts(rr, 128)], ident)
            nc.scalar.copy(out=V_sb[g][:, 4 * j:4 * j + 4, :],
                           in_=tp.rearrange("p (r t) -> p r t", r=4))
        return run

    def proj_closures(j, positions):
        """Closures for projection d-tiles at the given DT_ORDER positions."""
        cls = []
        for p in positions:
            dt = DT_ORDER[p]
            # prefetch weights two positions ahead (positions 0/1 of this
            # window were issued by the previous window / lead-in)
            if p + 2 < NDT:
                cls.append(issue_wt(j, p + 2))
            elif j + 1 < NJ:
                cls.append(issue_wt(j + 1, p + 2 - NDT))

            # matmul chunks of 4 k-tiles; PSUM tile created by first chunk
            st = {}

            def mk_chunk(p=p, k0=0, k1=0, st=st):
                def run():
                    if k0 == 0:
                        st['pp'] = ps.tile([128, SW], F32, tag="pp", bufs=2,
                                           name="pp")
                    wt = wt_tiles[(j, p)]
                    xt = xt_tiles[j]
                    for k in range(k0, k1):
                        nc.tensor.matmul(st['pp'], wt[:, k, :], xt[:, k, :],
                                         start=(k == 0), stop=(k == NKT - 1),
                                         skip_group_check=True)
                return run

            for k0 in range(0, NKT, 6):
                cls.append(mk_chunk(p=p, k0=k0, k1=k0 + 6, st=st))

            def mk_evac(dt=dt, st=st):
                def run():
                    if dt >= 8:
                        vraw = raw_pool.tile([128, SW], F32, name="vraw")
                        nc.any.tensor_copy(out=vraw, in_=st['pp'])
                        st['post'] = post_v(j, dt, vraw)
                    else:
                        raw = raw_pool.tile([128, SW], BF16, name="raw")
                        nc.any.tensor_copy(out=raw, in_=st['pp'])
                        sw_ps = ps.tile([128, SW], F32, tag="pp", bufs=2,
                                        name="sw_ps")
                        nc.tensor.matmul(sw_ps, swp_sb, raw, start=True,
                                         stop=True, skip_group_check=True)
                        st['post'] = post_qk(j, dt, raw, sw_ps)
                return run

            cls.append(mk_evac(dt=dt, st=st))
            # deferred post of this dt runs one position later; stash it
            cls.append(('post_marker', st))
        # resolve post markers: run each dt's post inside the NEXT dt's block
        out, pending = [], []
        for c in cls:
            if isinstance(c, tuple):
                pending.append(c[1])
                if len(pending) > 1:
                    stp = pending.pop(0)
                    out.append(lambda stp=stp: stp['post']())
            else:
                out.append(c)
        if pending:
            stp = pending.pop(0)
            out.append(lambda stp=stp: stp['post']())
        return out

    # ---- attention ----
    def pair_closures(j, units, l_on_pe=False, av_tags=None):
        n = len(units)
        if av_tags is None:
            av_tags = ("av",) * n
        lrows = tuple(32 * a for a in range(n))
        nlast = 4 * j + 3
        st = {}
        cls = []

        def start():
            if l_on_pe:
                st['lp'] = ps.tile([128, SW], F32, tag="l", bufs=1, name="lp")
            else:
                st['acc'] = [acc_pool.tile([128, SW], mybir.dt.float32r,
                                           name="acc") for _ in range(n)]
            st['avs'] = [ps.tile([128, SW], F32, tag=av_tags[a], bufs=2,
                                 name=f"av{a}") for a in range(n)]
            st['pr'] = {}
        cls.append(start)

        def _lo(i):
            # valid s-range of diagonal tile r starts at 128*r (causal)
            r = i - 4 * j
            return 128 * r if r > 0 else 0

        def emit_scores(i, a):
            u = units[a]
            g = u // 3
            lo = _lo(i)
            sc = ps.tile([128, SW], F32, tag="sy", bufs=3, name="sc")
            nc.tensor.matmul(sc[:, lo:], KT_sb[g][:, bass.ts(i, 128)],
                             qTj[j][u][:, lo:], start=True, stop=True,
                             skip_group_check=True)
            pr = probs_pool.tile([128, SW], BF16, name="pr")
            nc.scalar.activation(out=pr[:, lo:], in_=sc[:, lo:],
                                 func=mybir.ActivationFunctionType.Exp)
            if i >= 4 * j:
                # only the first 128 columns of the valid range straddle the
                # diagonal; the rest is fully visible
                nc.vector.tensor_mul(pr[:, lo:lo + 128], pr[:, lo:lo + 128],
                                     masks_sb[:, i - 4 * j, lo:lo + 128])
            st['pr'][(i, a)] = pr
            if not l_on_pe:
                if i == 0:
                    nc.scalar.copy(out=st['acc'][a], in_=pr)
                else:
                    nc.gpsimd.tensor_add(st['acc'][a][:, lo:],
                                         st['acc'][a][:, lo:], pr[:, lo:])

        def emit_lav(i, a):
            u = units[a]
            g = u // 3
            lo = _lo(i)
            pr = st['pr'].pop((i, a))
            if l_on_pe:
                row = lrows[a]
                nc.tensor.matmul(st['lp'][row:row + 1, lo:], ones_t,
                                 pr[:, lo:], start=(i == 0), stop=(i == nlast),
                                 skip_group_check=True)
            nc.tensor.matmul(st['avs'][a][:, lo:], V_sb[g][:, i, :],
                             pr[:, lo:], start=(i == 0), stop=(i == nlast),
                             skip_group_check=True)

        def mk_step(i):
            def run():
                for a in range(n):
                    emit_scores(i, a)
                if i >= 1:
                    for a in range(n):
                        emit_lav(i - 1, a)
                if i == nlast:
                    for a in range(n):
                        emit_lav(i, a)
            return run

        for i in range(4 * j + 4):
            cls.append(mk_step(i))

        def mk_fin(a):
            def run():
                u = units[a]
                l_sbp = small_pool.tile([1, SW], BF16, tag="rrow",
                                        name="l_sbp")
                if l_on_pe:
                    row = lrows[a]
                    nc.scalar.copy(out=l_sbp, in_=st['lp'][row:row + 1, :])
                else:
                    lc = ps.tile([128, SW], F32, tag="l", bufs=1, name="lc")
                    nc.tensor.matmul(lc[0:1, :], ones_r, st['acc'][a],
                                     start=True, stop=True,
                                     skip_group_check=True)
                    nc.scalar.copy(out=l_sbp, in_=lc[0:1, :])
                rb = ps.tile([128, SW], F32, tag="sy", bufs=3, name="rb")
                nc.tensor.matmul(rb, ones_mat[0:1, :], l_sbp, start=True,
                                 stop=True, skip_group_check=True)
                rbs = small_pool.tile([128, SW], F32, tag="rbs", name="rbs")
                nc.vector.reciprocal_approx_fast(out=rbs, in_=rb)
                ot = out_pool.tile([128, SW], BF16, name="ot")
                nc.vector.tensor_mul(ot, st['avs'][a], rbs)
                outTj[j][u] = ot
            return run

        for a in range(n):
            cls.append(mk_fin(a))
        return cls

    # ---- out-projection ----
    def outproj_closures(j):
        cls = []
        for d in range(NKT):
            def run(d=d):
                yp = ps.tile([128, SW], F32, tag="sy", bufs=3, name="yp")
                for u in range(QT_PER_CORE):
                    nc.tensor.matmul(yp, wo_sb[:, d, u, :], outTj[j][u],
                                     start=(u == 0),
                                     stop=(u == QT_PER_CORE - 1),
                                     skip_group_check=True)
                ys = y_pool.tile([128, SW], BF16, name="ys")
                nc.vector.tensor_copy(out=ys, in_=yp)
                nc.scalar.dma_start(out=yT[bass.ts(d, 128), bass.ts(j, SW)],
                                    in_=ys)
            cls.append(run)
        return cls

    # ---- schedule ----
    issue_lead_in()
    wo_cls = wo_load_closures()
    for j in range(NJ):
        kv_cls = proj_closures(j, [0, 1, 2, 3])
        q_cls = proj_closures(j, [4, 5, 6, 7, 8, 9])
        s2_extra = [issue_xt(j + 1)] if j + 1 < NJ else []
        if j == 0:
            _interleave(kv_cls, [])
            _interleave(q_cls + s2_extra, [])
            _interleave(wo_cls[:8], pair_closures(0, (0, 1)))
        else:
            prim1 = _splice(kv_cls, wo_cls[8:]) if j == 1 else kv_cls
            _interleave(prim1, pair_closures(j - 1, (2, 3)))
            _interleave(q_cls + s2_extra, pair_closures(j - 1, (4, 5)))
            _interleave(outproj_closures(j - 1), pair_closures(j, (0, 1)))
    # tail: last window's remaining units (3 on the PE row-sum path + 1 on
    # the gpsimd path), then its out-projection
    _interleave(pair_closures(NJ - 1, (2, 3, 4), l_on_pe=True,
                              av_tags=("av", "av", "pp")),
                pair_closures(NJ - 1, (5,), av_tags=("pp",)))
    for c in outproj_closures(NJ - 1):
        c()


def build_nc():
    nc = bacc.Bacc("TRN2", target_bir_lowering=False, debug=False, num_devices=8)
    io = {
        "x4": nc.dram_tensor("x4", [NJ, 128, NKT, SW], BF16, kind="ExternalInput"),
        "w10": nc.dram_tensor("w10", [NDT, 128, NKT, 128], BF16, kind="ExternalInput"),
        "wo4": nc.dram_tensor("wo4", [NKT, 128, QT_PER_CORE, 128], BF16,
                              kind="ExternalInput"),
        "ropeC": nc.dram_tensor("ropeC", [HD, S], F32, kind="ExternalInput"),
        "ropeS": nc.dram_tensor("ropeS", [HD, S], F32, kind="ExternalInput"),
        "masks": nc.dram_tensor("masks", [128, 4, SW], BF16, kind="ExternalInput"),
        "swp": nc.dram_tensor("swp", [128, 128], BF16, kind="ExternalInput"),
        "yT": nc.dram_tensor("yT", [DIM, S], BF16, kind="ExternalOutput"),
    }
    with tile.TileContext(nc) as tc:
        with ExitStack() as ctx:
            _build_body(nc, tc, io, ctx)
    nc.compile()
    return nc


_NC = None


def _get_nc():
    global _NC
    if _NC is None:
        _NC = build_nc()
    return _NC


def make_in_maps(x, wq, wk, wv, wo, freqs_cos, freqs_sin):
    x = np.asarray(x, np.float32)
    wq = np.asarray(wq, np.float32)
    wk = np.asarray(wk, np.float32)
    wv = np.asarray(wv, np.float32)
    wo = np.asarray(wo, np.float32)
    cos = np.asarray(freqs_cos, np.float32)
    sin = np.asarray(freqs_sin, np.float32)

    wq_p = (wq.reshape(DIM, NH, HD)[:, :, _PERM] * SCALE).astype(BF)
    wk_p = wk.reshape(DIM, NKV, HD)[:, :, _PERM].astype(BF)
    wv_r = wv.reshape(DIM, NKV, HD).astype(BF)
    wo_r = wo.reshape(NH, HD, DIM)

    ropeC = np.ascontiguousarray(np.concatenate([cos.T, cos.T], 0))
    ropeS = np.ascontiguousarray(np.concatenate([-sin.T, sin.T], 0))

    tt = np.arange(128)[:, None]
    ss = np.arange(SW)[None, :]
    # [128, 4, SW] with masks[:, r, :] the r-th diagonal-block pattern
    masks = np.stack([(128 * r + tt <= ss) for r in range(4)], axis=1).astype(BF)

    swp = np.zeros((128, 128), BF)
    swp[np.arange(128), (np.arange(128) + 64) % 128] = 1.0

    in_maps = []
    for c in range(8):
        b, p = divmod(c, 4)
        # per-core weight slices in on-chip tile layout
        wq_c = wq_p[:, 6 * p:6 * p + 6, :]          # [DIM, 6, 128]
        wk_c = wk_p[:, 2 * p:2 * p + 2, :]          # [DIM, 2, 128]
        wv_c = wv_r[:, 2 * p:2 * p + 2, :]          # [DIM, 2, 128]
        # w10[dt] = [128p, 24k, 128d] with DIM rows split as (k, p)
        wcat = np.concatenate([wq_c, wk_c, wv_c], axis=1)   # [DIM, 10, 128]
        w10 = np.ascontiguousarray(
            wcat.reshape(NKT, 128, NDT, HD).transpose(2, 1, 0, 3))
        # wo4[d] = [128p(dv), 6u, 128dd]; wo rows are (u, p)
        wo_c = wo_r[6 * p:6 * p + 6]                 # [6, 128, DIM]
        wo4 = np.ascontiguousarray(
            wo_c.reshape(QT_PER_CORE, HD, NKT, 128).transpose(2, 1, 0, 3)).astype(BF)
        # x4[j] = [128p, 24k, 512s]
        xT_b = x[b].T                                 # [DIM, S]
        x4 = np.ascontiguousarray(
            xT_b.reshape(NKT, 128, NJ, SW).transpose(2, 1, 0, 3)).astype(BF)
        in_maps.append({
            "x4": x4,
            "w10": w10,
            "wo4": wo4,
            "ropeC": ropeC,
            "ropeS": ropeS,
            "masks": masks,
            "swp": swp,
        })
    return in_maps


def gather(results):
    y = np.empty((B, S, DIM), np.float32)
    for b in range(B):
        acc = results[4 * b]["yT"].astype(np.float32)
        for p in range(1, 4):
            acc = acc + results[4 * b + p]["yT"].astype(np.float32)
        y[b] = acc.T
    return y


def kernel(x, wq, wk, wv, wo, freqs_cos, freqs_sin, **run_kwargs):
    nc = _get_nc()
    in_maps = make_in_maps(x, wq, wk, wv, wo, freqs_cos, freqs_sin)
    res = run_bass_kernel_spmd(nc, in_maps, core_ids=list(range(8)), **run_kwargs)
    out = gather(res.results)
    if run_kwargs:
        return out, res
    return out
